# revision 10
# baseline (speedup 1.0000x reference)
"""Trainium2 Bass kernel for nn_LoRALayer: out = x @ W.T + b + 2.0*(x@A.T)@B.T.

Strategy: 8-way data-parallel over the token dim (N=8192 -> 1024/core),
all-bf16 datapath (inputs host-cast to bf16; fp32 PSUM accumulation;
fp32 output):

  - x, W and lora_A are transposed into contraction-major (i-major)
    layout by the DMA XBAR (dma_start_transpose, 2-byte dtype), so the
    PE spends its cycles only on matmul streams.
  - psum orientation is [token, out_f]: evicted tiles DMA straight to
    the output with no transpose.
  - The bias and the LoRA term fold into each PSUM accumulation group
    as one extra K=17 matmul: lhsT = [2*(x@A.T).T ; ones],
    rhs = [B.T ; b].
"""

import os

import numpy as np

try:
    import concourse.bass as bass  # noqa: F401
except ImportError:  # pragma: no cover
    import sys

    sys.path.insert(0, "/opt/trn_rl_repo")
    import concourse.bass as bass  # noqa: F401

import ml_dtypes
import concourse.tile as tile
from concourse import bacc, mybir
from concourse.bass_utils import run_bass_kernel_spmd
from concourse.masks import make_identity

P = 128
N_CORES = 8
N_TOK = 8192
NT = N_TOK // N_CORES  # tokens per core (1024)
KD = 4096  # in_features (contraction)
OD = 4096  # out_features
R = 16
SCALING = 2.0

KT = KD // P  # 32 k-tiles
MT = NT // P  # 8 token tiles per core
NOP = 8  # out-feature panels
OPW = OD // NOP  # 512
KC = 4  # k-tiles per transpose-DMA chunk
NKC = KT // KC  # 8 chunks

F32 = mybir.dt.float32
BF16 = mybir.dt.bfloat16

_NC_CACHE = None


def _build():
    from contextlib import ExitStack

    nc = bacc.Bacc("TRN2", target_bir_lowering=False, debug=False,
                   num_devices=N_CORES)
    x_d = nc.dram_tensor("x", [NT, KD], BF16, kind="ExternalInput").ap()
    w_d = nc.dram_tensor("W", [OD, KD], BF16, kind="ExternalInput").ap()
    b_d = nc.dram_tensor("b", [OD], BF16, kind="ExternalInput").ap()
    a_d = nc.dram_tensor("lora_A", [R, KD], BF16, kind="ExternalInput").ap()
    bb_d = nc.dram_tensor("lora_B", [OD, R], BF16, kind="ExternalInput").ap()
    out_d = nc.dram_tensor("out", [NT, OD], F32, kind="ExternalOutput").ap()

    with tile.TileContext(nc) as tc, ExitStack() as ctx:
        const = ctx.enter_context(tc.tile_pool(name="const", bufs=1))
        xt_pool = ctx.enter_context(tc.tile_pool(name="xt", bufs=1))
        wp_pool = ctx.enter_context(tc.tile_pool(name="wp", bufs=2))
        small = ctx.enter_context(tc.tile_pool(name="small", bufs=1))
        osb_pool = ctx.enter_context(tc.tile_pool(name="osb", bufs=4))
        ps = ctx.enter_context(tc.tile_pool(name="ps", bufs=1, space="PSUM"))

        # ---- tiny loads first: B blocks, b row, identity ----
        # All XBAR transposes go on the single sync queue: issuing them
        # from two HWDGE queues concurrently corrupts the transposed
        # data on HW (the XBAR is a shared resource).
        ident = const.tile([P, P], BF16)
        make_identity(nc, ident[:])
        bsb = const.tile([P, KT, R], BF16, name="bsb")  # B (kb,p)-blocked
        nc.sync.dma_start(bsb[:], bb_d.rearrange("(kb p) r -> p kb r", p=P))
        btbT = const.tile([32, OD], BF16, name="btbT")  # [B.T ; b]
        nc.sync.dma_start(btbT[R:R + 1, :],
                            b_d.rearrange("(one o) -> one o", one=1))
        onesb = small.tile([1, NT], BF16, tag="ones")
        nc.any.memset(onesb[:], 1.0)
        t1sb = const.tile([32, NT], BF16, name="t1sb")  # [2*(x@A.T).T ; 1]
        nc.sync.dma_start(t1sb[R:R + 1, :], onesb[:])

        # ---- A -> aT [128i, k, r], scaled by 2 ----
        aT = small.tile([P, KT, R], BF16, tag="aT")
        nc.sync.dma_start_transpose(aT[:], a_d)
        aTs = small.tile([P, KT, R], BF16, tag="aTs")
        nc.scalar.mul(aTs[:], aT[:], SCALING)

        # ---- x -> xT [128i, h, k, t] via DMA XBAR (token halves) ----
        # Half-split so t1(h) and the first panel's groups only wait on
        # half the x traffic.
        xT = xt_pool.tile([P, 2, KT, 512], BF16, name="xT")
        XC = 8  # k-tiles per x-transpose chunk

        def issue_x_half(h):
            for c in range(KT // XC):
                nc.sync.dma_start_transpose(
                    xT[:, h, c * XC:(c + 1) * XC, :],
                    x_d[h * 512:(h + 1) * 512, c * XC * P:(c + 1) * XC * P])

        wp_tiles = {}
        WC = 8  # k-tiles per W-transpose chunk
        NWC = KT // WC  # 4 chunks per panel

        def issue_wp_chunk(op, c):
            wp = wp_tiles.get(op)
            if wp is None:
                wp = wp_pool.tile([P, KT, OPW], BF16, tag="wp",
                                  name=f"wp{op}")
                wp_tiles[op] = wp
            nc.sync.dma_start_transpose(
                wp[:, c * WC:(c + 1) * WC, :],
                w_d[op * OPW:(op + 1) * OPW, c * WC * P:(c + 1) * WC * P])

        # startup order: x half 0, W panel 0 (gates the first groups),
        # then x half 1 and W panel 1 land under panel-0 compute.
        issue_x_half(0)
        for c in range(NWC):
            issue_wp_chunk(0, c)
        issue_x_half(1)
        for c in range(NWC):
            issue_wp_chunk(1, c)

        # ---- PE prologue: btbT rows 0..15 via PE transposes of B ----
        for kb in range(KT):
            pt = ps.tile([R, P], BF16, tag="bt", bufs=1)
            nc.tensor.transpose(pt[:], bsb[:, kb, :], ident[:])
            nc.vector.tensor_copy(btbT[0:R, kb * P:(kb + 1) * P], pt[:])

        # ---- t1 rows 0..15 (half h): psum [16, 512] = (2A).T-major @ xT ----
        def compute_t1(h):
            pc = ps.tile([R, 512], F32, tag="t1", bufs=1)
            for k in range(KT):
                nc.tensor.matmul(pc[:], aTs[:, k, :], xT[:, h, k, :],
                                 start=(k == 0), stop=(k == KT - 1))
            nc.scalar.copy(t1sb[0:R, h * 512:(h + 1) * 512], pc[:])

        # ---- per (o-panel, token-tile) psum group of 33 matmuls ----
        def do_group(op, tt):
            h, t2 = divmod(tt, MT // 2)
            po = ps.tile([P, OPW], F32, tag="po", bufs=6)
            for k in range(KT):
                nc.tensor.matmul(po[:], xT[:, h, k, t2 * P:(t2 + 1) * P],
                                 wp_tiles[op][:, k, :],
                                 start=(k == 0), stop=False)
            nc.tensor.matmul(po[:], t1sb[0:R + 1, tt * P:(tt + 1) * P],
                             btbT[0:R + 1, op * OPW:(op + 1) * OPW],
                             start=False, stop=True)
            if 1 <= op < NOP - 1 and tt % 2 == 0:
                issue_wp_chunk(op + 1, tt // 2)
            osb = osb_pool.tile([P, OPW], F32, tag="osb")
            nc.scalar.copy(osb[:], po[:])
            nc.sync.dma_start(
                out_d[tt * P:(tt + 1) * P, op * OPW:(op + 1) * OPW],
                osb[:])

        # panel 0: half-0 groups after t1(0); t1(1) slots in while x half 1
        # lands; then half-1 groups. Panels 1+ run straight through.
        compute_t1(0)
        for tt in range(MT // 2):
            do_group(0, tt)
        compute_t1(1)
        for tt in range(MT // 2, MT):
            do_group(0, tt)
        for op in range(1, NOP):
            for tt in range(MT):
                do_group(op, tt)
            wp_tiles.pop(op - 1, None)

    nc.compile()
    return nc


def _get_nc():
    global _NC_CACHE
    if _NC_CACHE is None:
        _NC_CACHE = _build()
    return _NC_CACHE


def kernel(x, W, b, lora_A, lora_B):
    nc = _get_nc()
    bf = ml_dtypes.bfloat16
    x = np.ascontiguousarray(np.asarray(x, dtype=np.float32).astype(bf))
    W = np.ascontiguousarray(np.asarray(W, dtype=np.float32).astype(bf))
    b = np.ascontiguousarray(np.asarray(b, dtype=np.float32).astype(bf))
    lora_A = np.ascontiguousarray(
        np.asarray(lora_A, dtype=np.float32).astype(bf))
    lora_B = np.ascontiguousarray(
        np.asarray(lora_B, dtype=np.float32).astype(bf))
    in_maps = [
        {
            "x": x[c * NT:(c + 1) * NT],
            "W": W,
            "b": b,
            "lora_A": lora_A,
            "lora_B": lora_B,
        }
        for c in range(N_CORES)
    ]
    res = run_bass_kernel_spmd(nc, in_maps, core_ids=list(range(N_CORES)),
                               trace=bool(int(os.environ.get("LORA_TRACE", "0"))))
    kernel.last_results = res
    return np.concatenate([res.results[c]["out"] for c in range(N_CORES)],
                          axis=0)


if __name__ == "__main__":
    rng = np.random.default_rng(0)
    x = rng.standard_normal((N_TOK, KD), dtype=np.float32)
    W = (rng.standard_normal((OD, KD)) * 0.02).astype(np.float32)
    b = (rng.standard_normal(OD) * 0.02).astype(np.float32)
    A = (rng.standard_normal((R, KD)) * 0.02).astype(np.float32)
    B = (rng.standard_normal((OD, R)) * 0.02).astype(np.float32)
    out = kernel(x=x, W=W, b=b, lora_A=A, lora_B=B)
    ref = x.astype(np.float64) @ W.T.astype(np.float64) + b + SCALING * (
        (x.astype(np.float64) @ A.T.astype(np.float64)) @ B.T.astype(np.float64))
    rel = np.linalg.norm(out - ref) / np.linalg.norm(ref)
    print("rel_l2:", rel)


# revision 11
# speedup vs baseline: 1.1791x; 1.1791x over previous
"""Trainium2 Bass kernel for nn_LoRALayer: out = x @ W.T + b + 2.0*(x@A.T)@B.T.

Strategy: 8-way data-parallel over the token dim (N=8192 -> 1024/core),
all-bf16 datapath (inputs host-cast to bf16; fp32 PSUM accumulation;
fp32 output):

  - x, W and lora_A are transposed into contraction-major (i-major)
    layout by the DMA XBAR (dma_start_transpose, 2-byte dtype), so the
    PE spends its cycles only on matmul streams.
  - psum orientation is [token, out_f]: evicted tiles DMA straight to
    the output with no transpose.
  - The bias and the LoRA term fold into each PSUM accumulation group
    as one extra K=17 matmul: lhsT = [2*(x@A.T).T ; ones],
    rhs = [B.T ; b].
"""

import os

import numpy as np

try:
    import concourse.bass as bass  # noqa: F401
except ImportError:  # pragma: no cover
    import sys

    sys.path.insert(0, "/opt/trn_rl_repo")
    import concourse.bass as bass  # noqa: F401

import ml_dtypes
import concourse.tile as tile
from concourse import bacc, mybir
from concourse.bass_utils import run_bass_kernel_spmd
from concourse.masks import make_identity

P = 128
N_CORES = 8
N_TOK = 8192
NT = N_TOK // N_CORES  # tokens per core (1024)
KD = 4096  # in_features (contraction)
OD = 4096  # out_features
R = 16
SCALING = 2.0

KT = KD // P  # 32 k-tiles
MT = NT // P  # 8 token tiles per core
NOP = 8  # out-feature panels
OPW = OD // NOP  # 512
KC = 4  # k-tiles per transpose-DMA chunk
NKC = KT // KC  # 8 chunks

F32 = mybir.dt.float32
BF16 = mybir.dt.bfloat16

_NC_CACHE = None


def _build():
    from contextlib import ExitStack

    nc = bacc.Bacc("TRN2", target_bir_lowering=False, debug=False,
                   num_devices=N_CORES)
    x_d = nc.dram_tensor("x", [NT, KD], BF16, kind="ExternalInput").ap()
    w_d = nc.dram_tensor("W", [OD, KD], BF16, kind="ExternalInput").ap()
    b_d = nc.dram_tensor("b", [OD], BF16, kind="ExternalInput").ap()
    a_d = nc.dram_tensor("lora_A", [R, KD], BF16, kind="ExternalInput").ap()
    bb_d = nc.dram_tensor("lora_B", [OD, R], BF16, kind="ExternalInput").ap()
    out_d = nc.dram_tensor("out", [NT, OD], F32, kind="ExternalOutput").ap()

    with tile.TileContext(nc) as tc, ExitStack() as ctx:
        const = ctx.enter_context(tc.tile_pool(name="const", bufs=1))
        xt_pool = ctx.enter_context(tc.tile_pool(name="xt", bufs=1))
        wp_pool = ctx.enter_context(tc.tile_pool(name="wp", bufs=2))
        small = ctx.enter_context(tc.tile_pool(name="small", bufs=1))
        osb_pool = ctx.enter_context(tc.tile_pool(name="osb", bufs=6))
        ps = ctx.enter_context(tc.tile_pool(name="ps", bufs=1, space="PSUM"))

        # ---- tiny loads first: B blocks, b row, identity ----
        # All XBAR transposes go on the single sync queue: issuing them
        # from two HWDGE queues concurrently corrupts the transposed
        # data on HW (the XBAR is a shared resource). Plain DMAs (small
        # loads, evicts) go on scalar's queue so they never wait behind
        # a multi-us XBAR chunk.
        ident = const.tile([P, P], BF16)
        make_identity(nc, ident[:])
        bsb = const.tile([P, KT, R], BF16, name="bsb")  # B (kb,p)-blocked
        nc.scalar.dma_start(bsb[:], bb_d.rearrange("(kb p) r -> p kb r", p=P))
        btbT = const.tile([32, OD], BF16, name="btbT")  # [B.T ; b]
        nc.scalar.dma_start(btbT[R:R + 1, :],
                            b_d.rearrange("(one o) -> one o", one=1))
        onesb = small.tile([1, NT], BF16, tag="ones")
        nc.any.memset(onesb[:], 1.0)
        t1sb = const.tile([32, NT], BF16, name="t1sb")  # [2*(x@A.T).T ; 1]
        nc.scalar.dma_start(t1sb[R:R + 1, :], onesb[:])

        # ---- A -> aT [128i, k, r], scaled by 2 ----
        aT = small.tile([P, KT, R], BF16, tag="aT")
        nc.sync.dma_start_transpose(aT[:], a_d)
        aTs = small.tile([P, KT, R], BF16, tag="aTs")
        nc.scalar.mul(aTs[:], aT[:], SCALING)

        # ---- x -> xT [128i, h, k, t] via DMA XBAR (token halves) ----
        # Half-split so t1(h) and the first panel's groups only wait on
        # half the x traffic.
        xT = xt_pool.tile([P, 2, KT, 512], BF16, name="xT")
        XC = 8  # k-tiles per x-transpose chunk

        def issue_x_half(h):
            for c in range(KT // XC):
                nc.sync.dma_start_transpose(
                    xT[:, h, c * XC:(c + 1) * XC, :],
                    x_d[h * 512:(h + 1) * 512, c * XC * P:(c + 1) * XC * P])

        wp_tiles = {}
        WC = 8  # k-tiles per W-transpose chunk
        NWC = KT // WC  # 4 chunks per panel

        def issue_wp_chunk(op, c):
            wp = wp_tiles.get(op)
            if wp is None:
                wp = wp_pool.tile([P, KT, OPW], BF16, tag="wp",
                                  name=f"wp{op}")
                wp_tiles[op] = wp
            nc.sync.dma_start_transpose(
                wp[:, c * WC:(c + 1) * WC, :],
                w_d[op * OPW:(op + 1) * OPW, c * WC * P:(c + 1) * WC * P])

        # startup order: x half 0, W panel 0 (gates the first groups),
        # then x half 1 and W panel 1 land under panel-0 compute.
        issue_x_half(0)
        for c in range(NWC):
            issue_wp_chunk(0, c)
        issue_x_half(1)
        for c in range(NWC):
            issue_wp_chunk(1, c)

        # ---- PE prologue: btbT rows 0..15 via PE transposes of B ----
        for kb in range(KT):
            pt = ps.tile([R, P], BF16, tag="bt", bufs=1)
            nc.tensor.transpose(pt[:], bsb[:, kb, :], ident[:])
            nc.vector.tensor_copy(btbT[0:R, kb * P:(kb + 1) * P], pt[:])

        # ---- t1 rows 0..15 (half h): psum [16, 512] = (2A).T-major @ xT ----
        def compute_t1(h):
            pc = ps.tile([R, 512], F32, tag="t1", bufs=1)
            for k in range(KT):
                nc.tensor.matmul(pc[:], aTs[:, k, :], xT[:, h, k, :],
                                 start=(k == 0), stop=(k == KT - 1))
            nc.scalar.copy(t1sb[0:R, h * 512:(h + 1) * 512], pc[:])

        # ---- per (o-panel, token-tile) psum group of 33 matmuls ----
        def do_group(op, tt):
            h, t2 = divmod(tt, MT // 2)
            po = ps.tile([P, OPW], F32, tag="po", bufs=6)
            for k in range(KT):
                nc.tensor.matmul(po[:], xT[:, h, k, t2 * P:(t2 + 1) * P],
                                 wp_tiles[op][:, k, :],
                                 start=(k == 0), stop=False)
            nc.tensor.matmul(po[:], t1sb[0:R + 1, tt * P:(tt + 1) * P],
                             btbT[0:R + 1, op * OPW:(op + 1) * OPW],
                             start=False, stop=True)
            if 1 <= op < NOP - 1 and tt % 2 == 0:
                issue_wp_chunk(op + 1, tt // 2)
            osb = osb_pool.tile([P, OPW], F32, tag="osb")
            nc.scalar.copy(osb[:], po[:])
            nc.scalar.dma_start(
                out_d[tt * P:(tt + 1) * P, op * OPW:(op + 1) * OPW],
                osb[:])

        # panel 0: half-0 groups after t1(0); t1(1) slots in while x half 1
        # lands; then half-1 groups. Panels 1+ run straight through.
        compute_t1(0)
        for tt in range(MT // 2):
            do_group(0, tt)
        compute_t1(1)
        for tt in range(MT // 2, MT):
            do_group(0, tt)
        for op in range(1, NOP):
            for tt in range(MT):
                do_group(op, tt)
            wp_tiles.pop(op - 1, None)

    nc.compile()
    return nc


def _get_nc():
    global _NC_CACHE
    if _NC_CACHE is None:
        _NC_CACHE = _build()
    return _NC_CACHE


def kernel(x, W, b, lora_A, lora_B):
    nc = _get_nc()
    bf = ml_dtypes.bfloat16
    x = np.ascontiguousarray(np.asarray(x, dtype=np.float32).astype(bf))
    W = np.ascontiguousarray(np.asarray(W, dtype=np.float32).astype(bf))
    b = np.ascontiguousarray(np.asarray(b, dtype=np.float32).astype(bf))
    lora_A = np.ascontiguousarray(
        np.asarray(lora_A, dtype=np.float32).astype(bf))
    lora_B = np.ascontiguousarray(
        np.asarray(lora_B, dtype=np.float32).astype(bf))
    in_maps = [
        {
            "x": x[c * NT:(c + 1) * NT],
            "W": W,
            "b": b,
            "lora_A": lora_A,
            "lora_B": lora_B,
        }
        for c in range(N_CORES)
    ]
    res = run_bass_kernel_spmd(nc, in_maps, core_ids=list(range(N_CORES)),
                               trace=bool(int(os.environ.get("LORA_TRACE", "0"))))
    kernel.last_results = res
    return np.concatenate([res.results[c]["out"] for c in range(N_CORES)],
                          axis=0)


if __name__ == "__main__":
    rng = np.random.default_rng(0)
    x = rng.standard_normal((N_TOK, KD), dtype=np.float32)
    W = (rng.standard_normal((OD, KD)) * 0.02).astype(np.float32)
    b = (rng.standard_normal(OD) * 0.02).astype(np.float32)
    A = (rng.standard_normal((R, KD)) * 0.02).astype(np.float32)
    B = (rng.standard_normal((OD, R)) * 0.02).astype(np.float32)
    out = kernel(x=x, W=W, b=b, lora_A=A, lora_B=B)
    ref = x.astype(np.float64) @ W.T.astype(np.float64) + b + SCALING * (
        (x.astype(np.float64) @ A.T.astype(np.float64)) @ B.T.astype(np.float64))
    rel = np.linalg.norm(out - ref) / np.linalg.norm(ref)
    print("rel_l2:", rel)
